# revision 65
# baseline (speedup 1.0000x reference)
"""Host-side sharding/prep + Bass device program for nn_BBGRUDecoder.

Host does index manipulation / data layout only; the device kernel does all
model arithmetic.

v4 design:
- conv1 slot arrays carry the root feature as slot KSLOT (weight 1.0) and are
  shipped pre-transposed [SLOT_W, rows] so the conv1 matmul needs no
  on-device transpose and no tree-reduce.
- conv2 does NOT gather h1 rows (SWDGE gather costs ~10ns/row on gpsimd).
  Instead the host lays out each edge's SOURCE-node slot data edge-major
  ([SLOT_W, E2_slots]) and the device recomputes h1 per edge subtile with one
  extra matmul+relu. No gather, no compaction, no DRAM h1 tables.
- conv1 computes h1 only for local V tiles (root term), kept resident in SBUF.
- s2 (edge->dst scatter weights) shipped transposed [128, E2_slots] for wide
  contiguous loads; s3/pool_gid/inv_cnt/emb are single-DMA preloads.
"""
import numpy as np
import ml_dtypes

BF16 = np.dtype(ml_dtypes.bfloat16)
NC = 8
P = 128
KSLOT = 16       # conv1 in-edge slots per node (max in-degree 13)
KSLOT2 = 17      # + root slot
F_IN = 5
SLOT_W = KSLOT2 * F_IN   # 85
F1 = 128
F2 = 256
HID = 128
TR = 10          # rounds per shot
MAXG_TILE = 32   # max graphs per node-tile (pool S3 width)
OCT = 16         # conv1 tiles per input DMA


def _pack_groups(sizes, esizes, cap_items, cap_groups, cap_edges):
    """Greedy-pack consecutive groups (each <=cap_items items) into tiles of
    <=cap_items items, <=cap_groups groups, and <=cap_edges edges (the edge
    cap keeps per-tile conv2 subtile counts uniform across cores)."""
    tiles = []
    i = 0
    n = len(sizes)
    while i < n:
        items = 0
        edges = 0
        g = 0
        while (i + g < n and g < cap_groups
               and items + sizes[i + g] <= cap_items
               and edges + esizes[i + g] <= cap_edges):
            items += sizes[i + g]
            edges += esizes[i + g]
            g += 1
        assert g > 0, (f"group {i} size {sizes[i]}/{esizes[i]} exceeds caps "
                       f"{cap_items}/{cap_edges}")
        tiles.append((i, g, items))
        i += g
    return tiles


def prep(inputs):
    x = np.asarray(inputs["x"], np.float32)
    ei = np.asarray(inputs["edge_index"], np.int64)
    ea = np.asarray(inputs["edge_attr"], np.float32)
    bl = np.asarray(inputs["batch_labels"], np.int64)
    lm = np.asarray(inputs["label_map"], np.int64)
    B = int(inputs["B"])
    NN = x.shape[0]
    src_g, dst_g = ei[0], ei[1]
    shot_of, round_of = lm[:, 0], lm[:, 1]
    n_shot_core = (B + NC - 1) // NC          # 128 shots per core
    GCOLS = n_shot_core * TR                  # 1280 graph-columns per core
    deg = np.bincount(dst_g, minlength=NN)
    assert deg.max() <= KSLOT

    # ---- global conv1 slot data [NN, KSLOT2, F_IN]; slot KSLOT = root ----
    xs_all = np.zeros((NN, KSLOT2, F_IN), np.float32)
    ea_all = np.zeros((NN, KSLOT2, F_IN), np.float32)
    xs_all[:, KSLOT] = x
    ea_all[:, KSLOT] = 1.0
    order = np.argsort(dst_g, kind="stable")
    ds = dst_g[order]
    sl = np.arange(len(ds)) - np.searchsorted(ds, ds)   # slot within dst run
    xs_all[ds, sl] = x[src_g[order]]
    ea_all[ds, sl] = ea[order][:, None]
    xs_flat = xs_all.reshape(NN, SLOT_W).astype(BF16)
    ea_flat = ea_all.reshape(NN, SLOT_W).astype(BF16)

    node_g = bl
    node_core = (shot_of[node_g] % NC).astype(np.int64)

    cores = []
    for d in range(NC):
        V = np.nonzero(node_core == d)[0]          # ascending node ids
        gids, gstart, gcnt = np.unique(node_g[V], return_index=True, return_counts=True)
        s_idx = shot_of[gids] // NC
        # round-major: GRU per-step embT slices come out contiguous
        gcol = round_of[gids] * n_shot_core + s_idx
        # per-graph conv2 edge counts (in-edges of the graph's nodes)
        gedge = np.zeros(len(gids), np.int64)
        np.add.at(gedge, np.searchsorted(gids, node_g[V]), deg[V])
        tiles = _pack_groups(gcnt.tolist(), gedge.tolist(), P, MAXG_TILE, 384)
        NT_V = len(tiles)
        vpos = np.full(NN, -1, np.int64)
        packed_rows = []
        tile_graphs = []
        for (g0, ng, ni) in tiles:
            rows = []
            for k in range(g0, g0 + ng):
                rows.append(V[gstart[k]:gstart[k] + gcnt[k]])
            rows = np.concatenate(rows)
            packed_rows.append(rows)
            tile_graphs.append((gcol[g0:g0 + ng], gcnt[g0:g0 + ng]))
        for t, rows in enumerate(packed_rows):
            vpos[rows] = t * P + np.arange(len(rows))

        E = np.nonzero(node_core[dst_g] == d)[0]
        cores.append(dict(
            d=d, V=V, NT_V=NT_V, packed_rows=packed_rows,
            tile_graphs=tile_graphs, vpos=vpos,
            e_src=src_g[E], e_dst=dst_g[E], e_ea=ea[E],
            gids=gids, gcol=gcol, gcnt=gcnt,
        ))

    # ---- shared static shapes ----
    NT_V = max(c["NT_V"] for c in cores)
    V_pad = NT_V * P

    T_sub = np.zeros(NT_V, np.int64)
    for c in cores:
        for t in range(NT_V):
            if t < c["NT_V"]:
                ne = int(deg[c["packed_rows"][t]].sum())
            else:
                ne = 0
            T_sub[t] = max(T_sub[t], -(-ne // P) if ne else 1)
    E2_slots = int(T_sub.sum()) * P
    NW = -(-NT_V // 4)     # scatter windows (4 tiles each)

    meta = dict(NT_V=NT_V, V_pad=V_pad, T_sub=T_sub.tolist(),
                E2_slots=E2_slots, GCOLS=GCOLS, G_rows=-(-(GCOLS + 1) // P) * P,
                n_shot_core=n_shot_core, B=B, NW=NW)

    # ---- per-core padded arrays ----
    for c in cores:
        vpos = c["vpos"]
        # conv1 V slot data, transposed [SLOT_W, V_pad]
        xsV = np.zeros((V_pad, SLOT_W), BF16)
        eaV = np.zeros((V_pad, SLOT_W), BF16)
        for t, rows in enumerate(c["packed_rows"]):
            xsV[t * P:t * P + len(rows)] = xs_flat[rows]
            eaV[t * P:t * P + len(rows)] = ea_flat[rows]
        # interleave xs/ea per conv1 oct-group: one DMA loads both
        xsTV = xsV.T
        eaTV = eaV.T
        xe1 = np.empty((SLOT_W, 2 * V_pad), BF16)
        off = 0
        t0 = 0
        while t0 < NT_V:
            nt = min(OCT, NT_V - t0)
            n = nt * P
            xe1[:, off:off + n] = xsTV[:, t0 * P:t0 * P + n]
            xe1[:, off + n:off + 2 * n] = eaTV[:, t0 * P:t0 * P + n]
            off += 2 * n
            t0 += nt
        assert off == 2 * V_pad

        # conv2: edge-major src slot data + s2 scatter weights, per tile
        xs2 = np.zeros((E2_slots, SLOT_W), BF16)
        ea2 = np.zeros((E2_slots, SLOT_W), BF16)
        s2 = np.zeros((E2_slots // P, P, P), np.float32)
        st = 0
        for t in range(NT_V):
            nsub = int(T_sub[t])
            if t < c["NT_V"]:
                sel = np.nonzero((vpos[c["e_dst"]] >= t * P) &
                                 (vpos[c["e_dst"]] < t * P + P))[0]
                es, ed, ew = c["e_src"][sel], c["e_dst"][sel], c["e_ea"][sel]
                ne = len(es)
                xs2[st * P:st * P + ne] = xs_flat[es]
                ea2[st * P:st * P + ne] = ea_flat[es]
                loc = vpos[ed] - t * P
                s2[st + np.arange(ne) // P, np.arange(ne) % P, loc] = ew
            st += nsub
        assert st * P == E2_slots
        # interleave xs2/ea2 per conv2 plan group: one DMA loads both
        xs2T = xs2.T
        ea2T = ea2.T
        GG = 8
        ss = np.concatenate([[0], np.cumsum(T_sub)]).astype(int)
        plan = []
        t = 0
        while t < NT_V:
            te = t
            while te < NT_V and ss[te + 1] - ss[t] <= GG:
                te += 1
            plan.append((int(ss[t]), int(ss[te])))
            t = te
        xe2 = np.empty((SLOT_W, 2 * E2_slots), BF16)
        off = 0
        for (a, b) in plan:
            n = (b - a) * P
            xe2[:, off:off + n] = xs2T[:, a * P:b * P]
            xe2[:, off + n:off + 2 * n] = ea2T[:, a * P:b * P]
            off += 2 * n
        assert off == 2 * E2_slots
        s2T = np.ascontiguousarray(
            s2.transpose(1, 0, 2).reshape(P, E2_slots).astype(BF16))

        # pool S3 / graph ids / inv counts
        s3 = np.zeros((NT_V, P, MAXG_TILE), np.float32)
        pool_gid = np.full((NT_V, MAXG_TILE), meta["GCOLS"] + 100, np.int64)
        for t in range(c["NT_V"]):
            gcols, gcnts = c["tile_graphs"][t]
            off = 0
            for j, (gc, n) in enumerate(zip(gcols, gcnts)):
                s3[t, off:off + n, j] = 1.0 / n   # mean-pool normalization
                pool_gid[t, j] = gc
                off += n
        s3T = np.ascontiguousarray(s3.transpose(1, 0, 2).astype(BF16))
        pg_pad = np.full((NW * 4, MAXG_TILE), meta["GCOLS"] + 100, np.int64)
        pg_pad[:NT_V] = pool_gid
        pgT = np.ascontiguousarray(pg_pad.reshape(NW, P).T.astype(np.int32))

        amask = np.zeros(n_shot_core, np.float32)
        amask[(shot_of[c["gids"]] // NC)] = 1.0

        c["arrays"] = dict(
            xe1=np.ascontiguousarray(xe1), xe2=np.ascontiguousarray(xe2),
            s2T=s2T,
            s3T=s3T, pgT=pgT, amask=amask,
        )
    return cores, meta


# ======================================================
"""Bass/Tile device program (per-core SPMD)."""
import concourse.bass as bass
import concourse.bacc as bacc
import concourse.mybir as mybir
from concourse.tile import TileContext


BF = mybir.dt.bfloat16
FP = mybir.dt.float32
AF = mybir.ActivationFunctionType


def build(meta, num_devices=8):
    NT_V = meta["NT_V"]
    V_pad = meta["V_pad"]
    T_sub = meta["T_sub"]
    E2_slots = meta["E2_slots"]
    GCOLS, G_rows = meta["GCOLS"], meta["G_rows"]
    NSH = meta["n_shot_core"]
    NW = meta["NW"]
    NT_G = G_rows // P

    nc = bacc.Bacc("TRN2", target_bir_lowering=False, debug=False,
                   num_devices=num_devices)

    def inp(name, shape, dt):
        return nc.dram_tensor(name, shape, dt, kind="ExternalInput")

    xe1_d = inp("xe1", [SLOT_W, 2 * V_pad], BF)
    xe2_d = inp("xe2", [SLOT_W, 2 * E2_slots], BF)
    w1s_d = inp("w1s", [P, F1], BF)
    s2T_d = inp("s2T", [P, E2_slots], BF)
    s3T_d = inp("s3T", [P, NT_V, MAXG_TILE], BF)
    pgT_d = inp("pgT", [P, NW], mybir.dt.int32)
    amask_d = inp("amask", [12, NSH], FP)
    ident_d = inp("ident", [P, P], BF)
    wrel2_d = inp("wrel2", [P, F2], BF)
    wroot2_d = inp("wroot2", [P, F2], BF)
    wih0_d = inp("wih0", [3, 2, P, P], BF)
    whh0_d = inp("whh0", [3, P, P], BF)
    wih1_d = inp("wih1", [3, P, P], BF)
    whh1_d = inp("whh1", [3, P, P], BF)
    dec_d = inp("dec", [P, 12], BF)
    out_d = nc.dram_tensor("out", [12, NSH], FP, kind="ExternalOutput")

    emb_d = nc.dram_tensor("emb", [G_rows, F2], FP, kind="Internal")

    with TileContext(nc) as tc:
        with (
            tc.tile_pool(name="const", bufs=1) as cpool,
            tc.tile_pool(name="sb", bufs=3) as pool,
            tc.tile_pool(name="big", bufs=3) as bigp,
            tc.tile_pool(name="psA", bufs=3, space="PSUM") as psA,
            tc.tile_pool(name="psH", bufs=2, space="PSUM") as psH,
            tc.tile_pool(name="psP", bufs=1, space="PSUM") as psP,
            tc.tile_pool(name="psC", bufs=2, space="PSUM") as psC,
        ):
            # ---------------- constants / preloads ----------------
            ident = cpool.tile([P, P], BF, tag="ident")
            nc.sync.dma_start(out=ident[:], in_=ident_d[:])
            w1s = cpool.tile([P, F1], BF, tag="w1s")
            nc.sync.dma_start(out=w1s[:], in_=w1s_d[:])
            wrel2 = cpool.tile([P, F2], BF, tag="wrel2")
            nc.sync.dma_start(out=wrel2[:], in_=wrel2_d[:])
            wroot2 = cpool.tile([P, F2], BF, tag="wroot2")
            nc.sync.dma_start(out=wroot2[:], in_=wroot2_d[:])

            wih0 = []
            for gate in range(3):
                for k in range(2):
                    wt = cpool.tile([P, P], BF, tag=f"wih0_{gate}_{k}")
                    nc.sync.dma_start(out=wt[:], in_=wih0_d[gate, k])
                    wih0.append(wt)

            def load3(dram, nm):
                ts = []
                for i in range(3):
                    wt = cpool.tile([P, P], BF, tag=f"{nm}{i}")
                    nc.sync.dma_start(out=wt[:], in_=dram[i])
                    ts.append(wt)
                return ts

            whh0 = load3(whh0_d, "whh0")
            wih1 = load3(wih1_d, "wih1")
            whh1 = load3(whh1_d, "whh1")
            dec = cpool.tile([P, 12], BF, tag="dec")
            nc.sync.dma_start(out=dec[:], in_=dec_d[:])
            am = cpool.tile([12, NSH], FP, tag="am")
            nc.sync.dma_start(out=am[:], in_=amask_d[:])
            s3all = cpool.tile([P, NT_V, MAXG_TILE], BF, tag="s3all")
            nc.sync.dma_start(out=s3all[:], in_=s3T_d[:])
            pgall = cpool.tile([P, NW], mybir.dt.int32, tag="pgall")
            nc.sync.dma_start(out=pgall[:], in_=pgT_d[:])

            # zero emb via gpsimd queue so the later indirect scatters
            # (same SWDGE FIFO) are ordered after it without a barrier
            zt = cpool.tile([P, F2], FP, tag="zero")
            nc.gpsimd.memset(zt[:], 0.0)
            for t in range(NT_G):
                nc.gpsimd.dma_start(out=emb_d[t * P:(t + 1) * P, :], in_=zt[:])

            # h1^T of V tiles stays resident for conv2's root term
            # (features on partitions, node columns)
            h1TVall = cpool.tile([P, V_pad], BF, tag="h1TVall")

            # ---------------- conv1 over V tiles ----------------
            n_oct = (NT_V + OCT - 1) // OCT
            for o in range(n_oct):
                t0 = o * OCT
                nt = min(OCT, NT_V - t0)
                xe_t = pool.tile([SLOT_W, 2 * OCT * P], BF, tag="xe1")
                nc.sync.dma_start(
                    out=xe_t[:, :2 * nt * P],
                    in_=xe1_d[:, 2 * t0 * P:2 * (t0 + nt) * P])
                msgT = pool.tile([SLOT_W, OCT * P], BF, tag="msg1")
                nc.vector.tensor_mul(out=msgT[:, :nt * P], in0=xe_t[:, :nt * P],
                                     in1=xe_t[:, nt * P:2 * nt * P])
                for g0 in range(0, nt, 4):
                    tg = t0 + g0
                    ng = min(4, nt - g0)
                    h1p = psC.tile([P, 4 * P], FP, tag="pC")
                    nc.tensor.matmul(
                        h1p[:, :ng * P], lhsT=w1s[:SLOT_W, :],
                        rhs=msgT[:, g0 * P:(g0 + ng) * P],
                        start=True, stop=True)
                    dst = h1TVall[:, tg * P:(tg + ng) * P]
                    if (o + g0 // 4) % 2 == 0:
                        nc.scalar.activation(dst, h1p[:, :ng * P], AF.Relu)
                    else:
                        nc.vector.tensor_relu(out=dst, in_=h1p[:, :ng * P])

            # ---------------- conv2 + pool (gather-free) ----------------
            GG = 8
            sub_start = np.concatenate([[0], np.cumsum(T_sub)]).astype(int)
            plan = []
            t = 0
            while t < NT_V:
                te = t
                while te < NT_V and sub_start[te + 1] - sub_start[t] <= GG:
                    te += 1
                plan.append((t, te))
                t = te
            pool_ps = None
            for (ta, te) in plan:
                so0 = int(sub_start[ta])
                ns = int(sub_start[te]) - so0
                ecols = slice(so0 * P, (so0 + ns) * P)
                xe_t = pool.tile([SLOT_W, 2 * GG * P], BF, tag="xe2")
                nc.sync.dma_start(
                    out=xe_t[:, :2 * ns * P],
                    in_=xe2_d[:, 2 * so0 * P:2 * (so0 + ns) * P])
                msg2 = pool.tile([SLOT_W, GG * P], BF, tag="msg2")
                nc.vector.tensor_mul(out=msg2[:, :ns * P], in0=xe_t[:, :ns * P],
                                     in1=xe_t[:, ns * P:2 * ns * P])
                s2g = bigp.tile([P, GG, P], BF, tag="s2g")
                nc.sync.dma_start(
                    out=s2g[:, :ns, :],
                    in_=s2T_d[:, ecols].rearrange("p (s q) -> p s q", q=P))
                # h1 of edge sources, 4 subtiles per PSUM bank
                gts = bigp.tile([P, GG * F1], BF, tag="gts")
                for sb in range(0, ns, 4):
                    nb = min(4, ns - sb)
                    hep = psC.tile([P, 4 * P], FP, tag="pC")
                    for k in range(nb):
                        nc.tensor.matmul(
                            hep[:, k * P:(k + 1) * P],
                            lhsT=msg2[:, (sb + k) * P:(sb + k + 1) * P],
                            rhs=w1s[:SLOT_W, :],
                            start=True, stop=True)
                    dst = gts[:, sb * F1:(sb + nb) * F1]
                    if (sb // 4) % 2 == 0:
                        nc.scalar.activation(dst, hep[:, :nb * P], AF.Relu)
                    else:
                        nc.vector.tensor_relu(out=dst, in_=hep[:, :nb * P])
                for t in range(ta, te):
                    so = int(sub_start[t]) - so0
                    nsub = T_sub[t]
                    agg2T = psA.tile([P, P], FP, tag="pA")
                    for s in range(nsub):
                        nc.tensor.matmul(
                            agg2T[:], lhsT=gts[:, (so + s) * F1:(so + s + 1) * F1],
                            rhs=s2g[:, so + s, :],
                            start=(s == 0), stop=(s == nsub - 1))
                    agg2Ts = pool.tile([P, P], BF, tag="agg2Ts")
                    nc.vector.tensor_copy(out=agg2Ts[:], in_=agg2T[:])
                    h2p = psH.tile([P, F2], FP, tag="pB")
                    nc.tensor.matmul(h2p[:], lhsT=agg2Ts[:], rhs=wrel2[:],
                                     start=True, stop=False)
                    nc.tensor.matmul(h2p[:], lhsT=h1TVall[:, t * P:(t + 1) * P],
                                     rhs=wroot2[:], start=False, stop=True)
                    h2s = pool.tile([P, F2], BF, tag="h2s")
                    if t % 2 == 0:
                        nc.scalar.activation(h2s[:], h2p[:], AF.Relu)
                    else:
                        nc.vector.tensor_relu(out=h2s[:], in_=h2p[:])
                    jj = t % 4
                    if jj == 0:
                        pool_ps = psP.tile([P, F2], FP, tag="pP")
                    nc.tensor.matmul(
                        pool_ps[32 * jj:32 * jj + 32, :], lhsT=s3all[:, t, :],
                        rhs=h2s[:], start=True, stop=True,
                        tile_position=(0, 32 * jj))
                    if jj == 3 or t == NT_V - 1:
                        npart = 32 * (jj + 1)
                        w = t // 4
                        pls = pool.tile([P, F2], FP, tag="pls")
                        nc.vector.tensor_copy(out=pls[:npart, :],
                                              in_=pool_ps[:npart, :])
                        nc.gpsimd.indirect_dma_start(
                            out=emb_d[:, :],
                            out_offset=bass.IndirectOffsetOnAxis(
                                ap=pgall[:npart, w:w + 1], axis=0),
                            in_=pls[:npart, :], in_offset=None,
                            bounds_check=GCOLS, oob_is_err=False)

            tc.strict_bb_all_engine_barrier()

            # ---------------- emb -> embT ----------------
            # per-tile reloads: transpose t (and GRU step t, round-major)
            # starts as soon as its own tile lands
            emball = cpool.tile([P, NT_G, F2], FP, tag="emball")
            for t in range(NT_G):
                nc.sync.dma_start(out=emball[:, t, :],
                                  in_=emb_d[t * P:(t + 1) * P, :])
            embT0 = cpool.tile([P, G_rows], BF, tag="embT0")
            embT1 = cpool.tile([P, G_rows], BF, tag="embT1")
            for t in range(NT_G):
                etb = pool.tile([P, F2], BF, tag="etb")
                nc.vector.tensor_copy(out=etb[:], in_=emball[:, t, :])
                for half in range(2):
                    tp = psA.tile([P, P], FP, tag="pA")
                    nc.tensor.matmul(tp[:], lhsT=etb[:, half * P:(half + 1) * P],
                                     rhs=ident[:], start=True, stop=True)
                    dst = embT0 if half == 0 else embT1
                    nc.vector.tensor_copy(out=dst[:, t * P:(t + 1) * P], in_=tp[:])

            # ---------------- GRU ----------------
            # Both layers per-step with PSUM-accumulated gates ([P,512] =
            # r-sum, z-sum, i_n, h_n; i_n/h_n separate for the reset gate).
            # L1 runs interleaved one step behind L0, consuming h0 directly.
            h0 = cpool.tile([P, NSH], BF, tag="h_L0")
            nc.gpsimd.memset(h0[:], 0.0)
            h1 = cpool.tile([P, NSH], BF, tag="h_L1")
            nc.gpsimd.memset(h1[:], 0.0)
            for t in range(TR):
                # --- L0 step t (input = embT slices at step t) ---
                tc0 = slice(t * NSH, (t + 1) * NSH)
                ghp = psC.tile([P, 512], FP, tag="pC")
                for gate in range(2):   # r, z
                    dst = ghp[:, gate * P:(gate + 1) * P]
                    nc.tensor.matmul(dst, lhsT=wih0[gate * 2 + 0][:],
                                     rhs=embT0[:, tc0], start=True, stop=False)
                    nc.tensor.matmul(dst, lhsT=wih0[gate * 2 + 1][:],
                                     rhs=embT1[:, tc0], start=False, stop=False)
                    nc.tensor.matmul(dst, lhsT=whh0[gate][:], rhs=h0[:],
                                     start=False, stop=True)
                nc.tensor.matmul(ghp[:, 2 * P:3 * P], lhsT=wih0[4][:],
                                 rhs=embT0[:, tc0], start=True, stop=False)
                nc.tensor.matmul(ghp[:, 2 * P:3 * P], lhsT=wih0[5][:],
                                 rhs=embT1[:, tc0], start=False, stop=True)
                nc.tensor.matmul(ghp[:, 3 * P:4 * P], lhsT=whh0[2][:],
                                 rhs=h0[:], start=True, stop=True)
                rz0 = pool.tile([P, 2 * NSH], FP, tag="rs")
                nc.scalar.activation(rz0[:], ghp[:, 0:2 * P], AF.Sigmoid)
                ns_ = pool.tile([P, NSH], FP, tag="ns")
                nc.vector.tensor_mul(out=ns_[:], in0=rz0[:, 0:NSH],
                                     in1=ghp[:, 3 * P:4 * P])
                nc.vector.tensor_add(out=ns_[:], in0=ns_[:], in1=ghp[:, 2 * P:3 * P])
                nc.scalar.activation(ns_[:], ns_[:], AF.Tanh)
                hmn = pool.tile([P, NSH], FP, tag="hmn")
                nc.vector.tensor_sub(out=hmn[:], in0=h0[:], in1=ns_[:])
                nc.vector.tensor_mul(out=hmn[:], in0=hmn[:], in1=rz0[:, NSH:2 * NSH])
                nc.vector.tensor_add(out=h0[:], in0=ns_[:], in1=hmn[:])
                # --- L1 step t (input = updated h0) ---
                g1p = psC.tile([P, 512], FP, tag="pC")
                for gate in range(2):   # r, z: input+hidden summed in psum
                    nc.tensor.matmul(g1p[:, gate * P:(gate + 1) * P],
                                     lhsT=wih1[gate][:], rhs=h0[:],
                                     start=True, stop=False)
                    nc.tensor.matmul(g1p[:, gate * P:(gate + 1) * P],
                                     lhsT=whh1[gate][:], rhs=h1[:],
                                     start=False, stop=True)
                nc.tensor.matmul(g1p[:, 2 * P:3 * P], lhsT=wih1[2][:],
                                 rhs=h0[:], start=True, stop=True)
                nc.tensor.matmul(g1p[:, 3 * P:4 * P], lhsT=whh1[2][:],
                                 rhs=h1[:], start=True, stop=True)
                rz1 = pool.tile([P, 2 * NSH], FP, tag="rz1")
                nc.scalar.activation(rz1[:], g1p[:, 0:2 * P], AF.Sigmoid)
                n1 = pool.tile([P, NSH], FP, tag="n1")
                nc.vector.tensor_mul(out=n1[:], in0=rz1[:, 0:NSH],
                                     in1=g1p[:, 3 * P:4 * P])
                nc.vector.tensor_add(out=n1[:], in0=n1[:], in1=g1p[:, 2 * P:3 * P])
                nc.scalar.activation(n1[:], n1[:], AF.Tanh)
                hm1 = pool.tile([P, NSH], FP, tag="hm1")
                nc.vector.tensor_sub(out=hm1[:], in0=h1[:], in1=n1[:])
                nc.vector.tensor_mul(out=hm1[:], in0=hm1[:], in1=rz1[:, NSH:2 * NSH])
                nc.vector.tensor_add(out=h1[:], in0=n1[:], in1=hm1[:])
            hlast = h1

            lp = psA.tile([P, P], FP, tag="pA")
            nc.tensor.matmul(lp[:12, :NSH], lhsT=dec[:], rhs=hlast[:],
                             start=True, stop=True)
            lo = pool.tile([12, NSH], FP, tag="lo")
            nc.vector.tensor_mul(out=lo[:], in0=lp[:12, :NSH], in1=am[:])
            nc.sync.dma_start(out=out_d[:], in_=lo[:])

    nc.compile()
    return nc


def make_in_map(c, meta, W):
    """Per-core input arrays for run_bass_kernel_spmd."""
    A = c["arrays"]
    bf = lambda a: np.ascontiguousarray(a, dtype=BF16)
    f32 = lambda a: np.ascontiguousarray(a, dtype=np.float32)

    w1s = np.zeros((P, F1), np.float32)
    w1s[0:KSLOT * F_IN] = np.tile(f32(W["c1_wrel"]), (KSLOT, 1))
    w1s[KSLOT * F_IN:SLOT_W] = f32(W["c1_wroot"])
    wih0 = np.stack([np.stack([f32(W["w_ih0"])[g * P:(g + 1) * P, k * P:(k + 1) * P].T
                               for k in range(2)]) for g in range(3)])
    whh0 = np.stack([f32(W["w_hh0"])[g * P:(g + 1) * P, :].T for g in range(3)])
    wih1 = np.stack([f32(W["w_ih1"])[g * P:(g + 1) * P, :].T for g in range(3)])
    whh1 = np.stack([f32(W["w_hh1"])[g * P:(g + 1) * P, :].T for g in range(3)])
    amask = np.broadcast_to(A["amask"][None, :], (12, meta["n_shot_core"]))

    return {
        "xe1": A["xe1"],
        "xe2": A["xe2"],
        "w1s": bf(w1s),
        "s2T": A["s2T"],
        "s3T": A["s3T"],
        "pgT": A["pgT"],
        "amask": f32(amask),
        "ident": bf(np.eye(P, dtype=np.float32)),
        "wrel2": bf(W["c2_wrel"]),
        "wroot2": bf(W["c2_wroot"]),
        "wih0": bf(wih0),
        "whh0": bf(whh0),
        "wih1": bf(wih1),
        "whh1": bf(whh1),
        "dec": bf(W["dec_w"]),
    }


# ------------------------------------------------------------------
_CACHE = {}


def _get_nc(meta):
    key = (meta["NT_V"], meta["E2_slots"], meta["G_rows"],
           tuple(meta["T_sub"]))
    if key not in _CACHE:
        _CACHE[key] = build(meta, num_devices=NC)
    return _CACHE[key]


def kernel(**inputs):
    import sys as _sys
    if "/opt/trn_rl_repo" not in _sys.path:
        _sys.path.insert(0, "/opt/trn_rl_repo")
    from concourse.bass_utils import run_bass_kernel_spmd

    for k in ("c1_b", "c2_b", "b_ih0", "b_hh0", "b_ih1", "b_hh1", "dec_b",
              "empty_emb"):
        assert not np.any(np.asarray(inputs[k])), f"nonzero {k} unsupported"

    cores, meta = prep(inputs)
    W = {k: np.asarray(v, np.float32) for k, v in inputs.items()
         if k not in ("x", "edge_index", "edge_attr", "batch_labels",
                      "label_map", "B")}
    nc = _get_nc(meta)
    in_maps = [make_in_map(c, meta, W) for c in cores]
    res = None
    for attempt in range(6):
        try:
            res = run_bass_kernel_spmd(nc, in_maps, core_ids=list(range(NC)))
            break
        except Exception:
            if attempt == 5:
                raise
    global LAST_RES
    LAST_RES = res
    B = meta["B"]
    out = np.zeros((B, 12), np.float32)
    nsh = meta["n_shot_core"]
    for d in range(NC):
        lg = res.results[d]["out"]          # [12, nsh]
        s = d + NC * np.arange(nsh)
        out[s[s < B]] = lg.T[s < B]
    return out


# revision 67
# speedup vs baseline: 1.0771x; 1.0771x over previous
"""Host-side sharding/prep + Bass device program for nn_BBGRUDecoder.

Host does index manipulation / data layout only; the device kernel does all
model arithmetic.

v4 design:
- conv1 slot arrays carry the root feature as slot KSLOT (weight 1.0) and are
  shipped pre-transposed [SLOT_W, rows] so the conv1 matmul needs no
  on-device transpose and no tree-reduce.
- conv2 does NOT gather h1 rows (SWDGE gather costs ~10ns/row on gpsimd).
  Instead the host lays out each edge's SOURCE-node slot data edge-major
  ([SLOT_W, E2_slots]) and the device recomputes h1 per edge subtile with one
  extra matmul+relu. No gather, no compaction, no DRAM h1 tables.
- conv1 computes h1 only for local V tiles (root term), kept resident in SBUF.
- s2 (edge->dst scatter weights) shipped transposed [128, E2_slots] for wide
  contiguous loads; s3/pool_gid/inv_cnt/emb are single-DMA preloads.
"""
import numpy as np
import ml_dtypes

BF16 = np.dtype(ml_dtypes.bfloat16)
NC = 8
P = 128
KSLOT = 16       # conv1 in-edge slots per node (max in-degree 13)
KSLOT2 = 17      # + root slot
F_IN = 5
SLOT_W = KSLOT2 * F_IN   # 85
F1 = 128
F2 = 256
HID = 128
TR = 10          # rounds per shot
MAXG_TILE = 32   # max graphs per node-tile (pool S3 width)
OCT = 16         # conv1 tiles per input DMA


def _pack_groups(sizes, esizes, cap_items, cap_groups, cap_edges):
    """Greedy-pack consecutive groups (each <=cap_items items) into tiles of
    <=cap_items items, <=cap_groups groups, and <=cap_edges edges (the edge
    cap keeps per-tile conv2 subtile counts uniform across cores)."""
    tiles = []
    i = 0
    n = len(sizes)
    while i < n:
        items = 0
        edges = 0
        g = 0
        while (i + g < n and g < cap_groups
               and items + sizes[i + g] <= cap_items
               and edges + esizes[i + g] <= cap_edges):
            items += sizes[i + g]
            edges += esizes[i + g]
            g += 1
        assert g > 0, (f"group {i} size {sizes[i]}/{esizes[i]} exceeds caps "
                       f"{cap_items}/{cap_edges}")
        tiles.append((i, g, items))
        i += g
    return tiles


def prep(inputs):
    x = np.asarray(inputs["x"], np.float32)
    ei = np.asarray(inputs["edge_index"], np.int64)
    ea = np.asarray(inputs["edge_attr"], np.float32)
    bl = np.asarray(inputs["batch_labels"], np.int64)
    lm = np.asarray(inputs["label_map"], np.int64)
    B = int(inputs["B"])
    NN = x.shape[0]
    src_g, dst_g = ei[0], ei[1]
    shot_of, round_of = lm[:, 0], lm[:, 1]
    n_shot_core = (B + NC - 1) // NC          # 128 shots per core
    GCOLS = n_shot_core * TR                  # 1280 graph-columns per core
    deg = np.bincount(dst_g, minlength=NN)
    assert deg.max() <= KSLOT

    # ---- global conv1 slot data [NN, KSLOT2, F_IN]; slot KSLOT = root ----
    xs_all = np.zeros((NN, KSLOT2, F_IN), np.float32)
    ea_all = np.zeros((NN, KSLOT2, F_IN), np.float32)
    xs_all[:, KSLOT] = x
    ea_all[:, KSLOT] = 1.0
    order = np.argsort(dst_g, kind="stable")
    ds = dst_g[order]
    sl = np.arange(len(ds)) - np.searchsorted(ds, ds)   # slot within dst run
    xs_all[ds, sl] = x[src_g[order]]
    ea_all[ds, sl] = ea[order][:, None]
    xs_flat = xs_all.reshape(NN, SLOT_W).astype(BF16)
    ea_flat = ea_all.reshape(NN, SLOT_W).astype(BF16)

    node_g = bl
    node_core = (shot_of[node_g] % NC).astype(np.int64)

    cores = []
    for d in range(NC):
        V = np.nonzero(node_core == d)[0]          # ascending node ids
        gids, gstart, gcnt = np.unique(node_g[V], return_index=True, return_counts=True)
        s_idx = shot_of[gids] // NC
        # round-major: GRU per-step embT slices come out contiguous
        gcol = round_of[gids] * n_shot_core + s_idx
        # per-graph conv2 edge counts (in-edges of the graph's nodes)
        gedge = np.zeros(len(gids), np.int64)
        np.add.at(gedge, np.searchsorted(gids, node_g[V]), deg[V])
        tiles = _pack_groups(gcnt.tolist(), gedge.tolist(), P, MAXG_TILE, 384)
        NT_V = len(tiles)
        vpos = np.full(NN, -1, np.int64)
        packed_rows = []
        tile_graphs = []
        for (g0, ng, ni) in tiles:
            rows = []
            for k in range(g0, g0 + ng):
                rows.append(V[gstart[k]:gstart[k] + gcnt[k]])
            rows = np.concatenate(rows)
            packed_rows.append(rows)
            tile_graphs.append((gcol[g0:g0 + ng], gcnt[g0:g0 + ng]))
        for t, rows in enumerate(packed_rows):
            vpos[rows] = t * P + np.arange(len(rows))

        E = np.nonzero(node_core[dst_g] == d)[0]
        cores.append(dict(
            d=d, V=V, NT_V=NT_V, packed_rows=packed_rows,
            tile_graphs=tile_graphs, vpos=vpos,
            e_src=src_g[E], e_dst=dst_g[E], e_ea=ea[E],
            gids=gids, gcol=gcol, gcnt=gcnt,
        ))

    # ---- shared static shapes ----
    NT_V = max(c["NT_V"] for c in cores)
    V_pad = NT_V * P

    T_sub = np.zeros(NT_V, np.int64)
    for c in cores:
        for t in range(NT_V):
            if t < c["NT_V"]:
                ne = int(deg[c["packed_rows"][t]].sum())
            else:
                ne = 0
            T_sub[t] = max(T_sub[t], -(-ne // P) if ne else 1)
    E2_slots = int(T_sub.sum()) * P
    NW = -(-NT_V // 4)     # scatter windows (4 tiles each)

    meta = dict(NT_V=NT_V, V_pad=V_pad, T_sub=T_sub.tolist(),
                E2_slots=E2_slots, GCOLS=GCOLS, G_rows=-(-(GCOLS + 1) // P) * P,
                n_shot_core=n_shot_core, B=B, NW=NW)

    # ---- per-core padded arrays ----
    for c in cores:
        vpos = c["vpos"]
        # conv1 V slot data, transposed [SLOT_W, V_pad]
        xsV = np.zeros((V_pad, SLOT_W), BF16)
        eaV = np.zeros((V_pad, SLOT_W), BF16)
        for t, rows in enumerate(c["packed_rows"]):
            xsV[t * P:t * P + len(rows)] = xs_flat[rows]
            eaV[t * P:t * P + len(rows)] = ea_flat[rows]
        # interleave xs/ea per conv1 oct-group: one DMA loads both
        xsTV = xsV.T
        eaTV = eaV.T
        xe1 = np.empty((SLOT_W, 2 * V_pad), BF16)
        off = 0
        t0 = 0
        while t0 < NT_V:
            nt = min(OCT, NT_V - t0)
            n = nt * P
            xe1[:, off:off + n] = xsTV[:, t0 * P:t0 * P + n]
            xe1[:, off + n:off + 2 * n] = eaTV[:, t0 * P:t0 * P + n]
            off += 2 * n
            t0 += nt
        assert off == 2 * V_pad

        # conv2: edge-major src slot data + s2 scatter weights, per tile
        xs2 = np.zeros((E2_slots, SLOT_W), BF16)
        ea2 = np.zeros((E2_slots, SLOT_W), BF16)
        s2 = np.zeros((E2_slots // P, P, P), np.float32)
        st = 0
        for t in range(NT_V):
            nsub = int(T_sub[t])
            if t < c["NT_V"]:
                sel = np.nonzero((vpos[c["e_dst"]] >= t * P) &
                                 (vpos[c["e_dst"]] < t * P + P))[0]
                es, ed, ew = c["e_src"][sel], c["e_dst"][sel], c["e_ea"][sel]
                ne = len(es)
                xs2[st * P:st * P + ne] = xs_flat[es]
                ea2[st * P:st * P + ne] = ea_flat[es]
                loc = vpos[ed] - t * P
                s2[st + np.arange(ne) // P, np.arange(ne) % P, loc] = ew
            st += nsub
        assert st * P == E2_slots
        # interleave xs2/ea2 per conv2 plan group: one DMA loads both
        xs2T = xs2.T
        ea2T = ea2.T
        GG = 8
        ss = np.concatenate([[0], np.cumsum(T_sub)]).astype(int)
        plan = []
        t = 0
        while t < NT_V:
            te = t
            while te < NT_V and ss[te + 1] - ss[t] <= GG:
                te += 1
            plan.append((int(ss[t]), int(ss[te])))
            t = te
        xe2 = np.empty((SLOT_W, 2 * E2_slots), BF16)
        off = 0
        for (a, b) in plan:
            n = (b - a) * P
            xe2[:, off:off + n] = xs2T[:, a * P:b * P]
            xe2[:, off + n:off + 2 * n] = ea2T[:, a * P:b * P]
            off += 2 * n
        assert off == 2 * E2_slots
        s2T = np.ascontiguousarray(
            s2.transpose(1, 0, 2).reshape(P, E2_slots).astype(BF16))

        # pool S3 / graph ids / inv counts
        s3 = np.zeros((NT_V, P, MAXG_TILE), np.float32)
        pool_gid = np.full((NT_V, MAXG_TILE), meta["GCOLS"] + 100, np.int64)
        for t in range(c["NT_V"]):
            gcols, gcnts = c["tile_graphs"][t]
            off = 0
            for j, (gc, n) in enumerate(zip(gcols, gcnts)):
                s3[t, off:off + n, j] = 1.0 / n   # mean-pool normalization
                pool_gid[t, j] = gc
                off += n
        s3T = np.ascontiguousarray(s3.transpose(1, 0, 2).astype(BF16))
        pg_pad = np.full((NW * 4, MAXG_TILE), meta["GCOLS"] + 100, np.int64)
        pg_pad[:NT_V] = pool_gid
        pgT = np.ascontiguousarray(pg_pad.reshape(NW, P).T.astype(np.int32))

        amask = np.zeros(n_shot_core, np.float32)
        amask[(shot_of[c["gids"]] // NC)] = 1.0

        c["arrays"] = dict(
            xe1=np.ascontiguousarray(xe1), xe2=np.ascontiguousarray(xe2),
            s2T=s2T,
            s3T=s3T, pgT=pgT, amask=amask,
        )
    return cores, meta


# ======================================================
"""Bass/Tile device program (per-core SPMD)."""
import concourse.bass as bass
import concourse.bacc as bacc
import concourse.mybir as mybir
from concourse.tile import TileContext


BF = mybir.dt.bfloat16
FP = mybir.dt.float32
AF = mybir.ActivationFunctionType


def build(meta, num_devices=8):
    NT_V = meta["NT_V"]
    V_pad = meta["V_pad"]
    T_sub = meta["T_sub"]
    E2_slots = meta["E2_slots"]
    GCOLS, G_rows = meta["GCOLS"], meta["G_rows"]
    NSH = meta["n_shot_core"]
    NW = meta["NW"]
    NT_G = G_rows // P

    nc = bacc.Bacc("TRN2", target_bir_lowering=False, debug=False,
                   num_devices=num_devices)

    def inp(name, shape, dt):
        return nc.dram_tensor(name, shape, dt, kind="ExternalInput")

    xe1_d = inp("xe1", [SLOT_W, 2 * V_pad], BF)
    xe2_d = inp("xe2", [SLOT_W, 2 * E2_slots], BF)
    w1s_d = inp("w1s", [P, F1], BF)
    s2T_d = inp("s2T", [P, E2_slots], BF)
    s3T_d = inp("s3T", [P, NT_V, MAXG_TILE], BF)
    pgT_d = inp("pgT", [P, NW], mybir.dt.int32)
    amask_d = inp("amask", [12, NSH], FP)
    ident_d = inp("ident", [P, P], BF)
    wrel2_d = inp("wrel2", [P, F2], BF)
    wroot2_d = inp("wroot2", [P, F2], BF)
    wih0_d = inp("wih0", [3, 2, P, P], BF)
    whh0_d = inp("whh0", [3, P, P], BF)
    wih1_d = inp("wih1", [3, P, P], BF)
    whh1_d = inp("whh1", [3, P, P], BF)
    dec_d = inp("dec", [P, 12], BF)
    out_d = nc.dram_tensor("out", [12, NSH], FP, kind="ExternalOutput")

    emb_d = nc.dram_tensor("emb", [G_rows, F2], FP, kind="Internal")

    with TileContext(nc) as tc:
        with (
            tc.tile_pool(name="const", bufs=1) as cpool,
            tc.tile_pool(name="sb", bufs=3) as pool,
            tc.tile_pool(name="big", bufs=3) as bigp,
            tc.tile_pool(name="psA", bufs=2, space="PSUM") as psA,
            tc.tile_pool(name="psH", bufs=3, space="PSUM") as psH,
            tc.tile_pool(name="psP", bufs=1, space="PSUM") as psP,
            tc.tile_pool(name="psC", bufs=2, space="PSUM") as psC,
        ):
            # ---------------- constants / preloads ----------------
            ident = cpool.tile([P, P], BF, tag="ident")
            nc.sync.dma_start(out=ident[:], in_=ident_d[:])
            w1s = cpool.tile([P, F1], BF, tag="w1s")
            nc.sync.dma_start(out=w1s[:], in_=w1s_d[:])
            wrel2 = cpool.tile([P, F2], BF, tag="wrel2")
            nc.sync.dma_start(out=wrel2[:], in_=wrel2_d[:])
            wroot2 = cpool.tile([P, F2], BF, tag="wroot2")
            nc.sync.dma_start(out=wroot2[:], in_=wroot2_d[:])

            wih0 = []
            for gate in range(3):
                for k in range(2):
                    wt = cpool.tile([P, P], BF, tag=f"wih0_{gate}_{k}")
                    nc.sync.dma_start(out=wt[:], in_=wih0_d[gate, k])
                    wih0.append(wt)

            def load3(dram, nm):
                ts = []
                for i in range(3):
                    wt = cpool.tile([P, P], BF, tag=f"{nm}{i}")
                    nc.sync.dma_start(out=wt[:], in_=dram[i])
                    ts.append(wt)
                return ts

            whh0 = load3(whh0_d, "whh0")
            wih1 = load3(wih1_d, "wih1")
            whh1 = load3(whh1_d, "whh1")
            dec = cpool.tile([P, 12], BF, tag="dec")
            nc.sync.dma_start(out=dec[:], in_=dec_d[:])
            am = cpool.tile([12, NSH], FP, tag="am")
            nc.sync.dma_start(out=am[:], in_=amask_d[:])
            s3all = cpool.tile([P, NT_V, MAXG_TILE], BF, tag="s3all")
            nc.sync.dma_start(out=s3all[:], in_=s3T_d[:])
            pgall = cpool.tile([P, NW], mybir.dt.int32, tag="pgall")
            nc.sync.dma_start(out=pgall[:], in_=pgT_d[:])

            # zero emb via gpsimd queue so the later indirect scatters
            # (same SWDGE FIFO) are ordered after it without a barrier
            zt = cpool.tile([P, F2], FP, tag="zero")
            nc.gpsimd.memset(zt[:], 0.0)
            for t in range(NT_G):
                nc.gpsimd.dma_start(out=emb_d[t * P:(t + 1) * P, :], in_=zt[:])

            # h1^T of V tiles stays resident for conv2's root term
            # (features on partitions, node columns)
            h1TVall = cpool.tile([P, V_pad], BF, tag="h1TVall")

            # ---------------- conv1 over V tiles ----------------
            n_oct = (NT_V + OCT - 1) // OCT
            for o in range(n_oct):
                t0 = o * OCT
                nt = min(OCT, NT_V - t0)
                xe_t = pool.tile([SLOT_W, 2 * OCT * P], BF, tag="xe1")
                nc.sync.dma_start(
                    out=xe_t[:, :2 * nt * P],
                    in_=xe1_d[:, 2 * t0 * P:2 * (t0 + nt) * P])
                msgT = pool.tile([SLOT_W, OCT * P], BF, tag="msg1")
                nc.vector.tensor_mul(out=msgT[:, :nt * P], in0=xe_t[:, :nt * P],
                                     in1=xe_t[:, nt * P:2 * nt * P])
                for g0 in range(0, nt, 4):
                    tg = t0 + g0
                    ng = min(4, nt - g0)
                    h1p = psC.tile([P, 4 * P], FP, tag="pC")
                    nc.tensor.matmul(
                        h1p[:, :ng * P], lhsT=w1s[:SLOT_W, :],
                        rhs=msgT[:, g0 * P:(g0 + ng) * P],
                        start=True, stop=True)
                    dst = h1TVall[:, tg * P:(tg + ng) * P]
                    if (o + g0 // 4) % 2 == 0:
                        nc.scalar.activation(dst, h1p[:, :ng * P], AF.Relu)
                    else:
                        nc.vector.tensor_relu(out=dst, in_=h1p[:, :ng * P])

            # ---------------- conv2 + pool (gather-free) ----------------
            GG = 8
            sub_start = np.concatenate([[0], np.cumsum(T_sub)]).astype(int)
            plan = []
            t = 0
            while t < NT_V:
                te = t
                while te < NT_V and sub_start[te + 1] - sub_start[t] <= GG:
                    te += 1
                plan.append((t, te))
                t = te
            pool_ps = None
            for (ta, te) in plan:
                so0 = int(sub_start[ta])
                ns = int(sub_start[te]) - so0
                ecols = slice(so0 * P, (so0 + ns) * P)
                xe_t = pool.tile([SLOT_W, 2 * GG * P], BF, tag="xe2")
                nc.sync.dma_start(
                    out=xe_t[:, :2 * ns * P],
                    in_=xe2_d[:, 2 * so0 * P:2 * (so0 + ns) * P])
                msg2 = pool.tile([SLOT_W, GG * P], BF, tag="msg2")
                nc.vector.tensor_mul(out=msg2[:, :ns * P], in0=xe_t[:, :ns * P],
                                     in1=xe_t[:, ns * P:2 * ns * P])
                s2g = bigp.tile([P, GG, P], BF, tag="s2g")
                nc.sync.dma_start(
                    out=s2g[:, :ns, :],
                    in_=s2T_d[:, ecols].rearrange("p (s q) -> p s q", q=P))
                # h1 of edge sources, 4 subtiles per PSUM bank
                gts = bigp.tile([P, GG * F1], BF, tag="gts")
                for sb in range(0, ns, 4):
                    nb = min(4, ns - sb)
                    hep = psC.tile([P, 4 * P], FP, tag="pC")
                    for k in range(nb):
                        nc.tensor.matmul(
                            hep[:, k * P:(k + 1) * P],
                            lhsT=msg2[:, (sb + k) * P:(sb + k + 1) * P],
                            rhs=w1s[:SLOT_W, :],
                            start=True, stop=True)
                    dst = gts[:, sb * F1:(sb + nb) * F1]
                    if (sb // 4) % 2 == 0:
                        nc.scalar.activation(dst, hep[:, :nb * P], AF.Relu)
                    else:
                        nc.vector.tensor_relu(out=dst, in_=hep[:, :nb * P])
                for t in range(ta, te):
                    so = int(sub_start[t]) - so0
                    nsub = T_sub[t]
                    agg2T = psA.tile([P, P], FP, tag="pA")
                    for s in range(nsub):
                        nc.tensor.matmul(
                            agg2T[:], lhsT=gts[:, (so + s) * F1:(so + s + 1) * F1],
                            rhs=s2g[:, so + s, :],
                            start=(s == 0), stop=(s == nsub - 1))
                    agg2Ts = pool.tile([P, P], BF, tag="agg2Ts")
                    nc.vector.tensor_copy(out=agg2Ts[:], in_=agg2T[:])
                    h2p = psH.tile([P, F2], FP, tag="pB")
                    nc.tensor.matmul(h2p[:], lhsT=agg2Ts[:], rhs=wrel2[:],
                                     start=True, stop=False)
                    nc.tensor.matmul(h2p[:], lhsT=h1TVall[:, t * P:(t + 1) * P],
                                     rhs=wroot2[:], start=False, stop=True)
                    h2s = pool.tile([P, F2], BF, tag="h2s")
                    if t % 2 == 0:
                        nc.scalar.activation(h2s[:], h2p[:], AF.Relu)
                    else:
                        nc.vector.tensor_relu(out=h2s[:], in_=h2p[:])
                    jj = t % 4
                    if jj == 0:
                        pool_ps = psP.tile([P, F2], FP, tag="pP")
                    nc.tensor.matmul(
                        pool_ps[32 * jj:32 * jj + 32, :], lhsT=s3all[:, t, :],
                        rhs=h2s[:], start=True, stop=True,
                        tile_position=(0, 32 * jj))
                    if jj == 3 or t == NT_V - 1:
                        npart = 32 * (jj + 1)
                        w = t // 4
                        pls = pool.tile([P, F2], FP, tag="pls")
                        nc.vector.tensor_copy(out=pls[:npart, :],
                                              in_=pool_ps[:npart, :])
                        nc.gpsimd.indirect_dma_start(
                            out=emb_d[:, :],
                            out_offset=bass.IndirectOffsetOnAxis(
                                ap=pgall[:npart, w:w + 1], axis=0),
                            in_=pls[:npart, :], in_offset=None,
                            bounds_check=GCOLS, oob_is_err=False)

            tc.strict_bb_all_engine_barrier()

            # ---------------- emb -> embT ----------------
            emball = cpool.tile([P, NT_G, F2], FP, tag="emball")
            nc.sync.dma_start(
                out=emball[:],
                in_=emb_d[:].rearrange("(t p) f -> p t f", p=P))
            embT0 = cpool.tile([P, G_rows], BF, tag="embT0")
            embT1 = cpool.tile([P, G_rows], BF, tag="embT1")
            for t in range(NT_G):
                etb = pool.tile([P, F2], BF, tag="etb")
                nc.vector.tensor_copy(out=etb[:], in_=emball[:, t, :])
                for half in range(2):
                    tp = psA.tile([P, P], FP, tag="pA")
                    nc.tensor.matmul(tp[:], lhsT=etb[:, half * P:(half + 1) * P],
                                     rhs=ident[:], start=True, stop=True)
                    dst = embT0 if half == 0 else embT1
                    nc.vector.tensor_copy(out=dst[:, t * P:(t + 1) * P], in_=tp[:])

            # ---------------- GRU ----------------
            # Both layers per-step with PSUM-accumulated gates ([P,512] =
            # r-sum, z-sum, i_n, h_n; i_n/h_n separate for the reset gate).
            # L1 runs interleaved one step behind L0, consuming h0 directly.
            h0 = cpool.tile([P, NSH], BF, tag="h_L0")
            nc.gpsimd.memset(h0[:], 0.0)
            h1 = cpool.tile([P, NSH], BF, tag="h_L1")
            nc.gpsimd.memset(h1[:], 0.0)
            for t in range(TR):
                # --- L0 step t (input = embT slices at step t) ---
                tc0 = slice(t * NSH, (t + 1) * NSH)
                ghp = psC.tile([P, 512], FP, tag="pC")
                for gate in range(2):   # r, z
                    dst = ghp[:, gate * P:(gate + 1) * P]
                    nc.tensor.matmul(dst, lhsT=wih0[gate * 2 + 0][:],
                                     rhs=embT0[:, tc0], start=True, stop=False)
                    nc.tensor.matmul(dst, lhsT=wih0[gate * 2 + 1][:],
                                     rhs=embT1[:, tc0], start=False, stop=False)
                    nc.tensor.matmul(dst, lhsT=whh0[gate][:], rhs=h0[:],
                                     start=False, stop=True)
                nc.tensor.matmul(ghp[:, 2 * P:3 * P], lhsT=wih0[4][:],
                                 rhs=embT0[:, tc0], start=True, stop=False)
                nc.tensor.matmul(ghp[:, 2 * P:3 * P], lhsT=wih0[5][:],
                                 rhs=embT1[:, tc0], start=False, stop=True)
                nc.tensor.matmul(ghp[:, 3 * P:4 * P], lhsT=whh0[2][:],
                                 rhs=h0[:], start=True, stop=True)
                rz0 = pool.tile([P, 2 * NSH], FP, tag="rs")
                nc.scalar.activation(rz0[:], ghp[:, 0:2 * P], AF.Sigmoid)
                ns_ = pool.tile([P, NSH], FP, tag="ns")
                nc.vector.tensor_mul(out=ns_[:], in0=rz0[:, 0:NSH],
                                     in1=ghp[:, 3 * P:4 * P])
                nc.vector.tensor_add(out=ns_[:], in0=ns_[:], in1=ghp[:, 2 * P:3 * P])
                nc.scalar.activation(ns_[:], ns_[:], AF.Tanh)
                hmn = pool.tile([P, NSH], FP, tag="hmn")
                nc.vector.tensor_sub(out=hmn[:], in0=h0[:], in1=ns_[:])
                nc.vector.tensor_mul(out=hmn[:], in0=hmn[:], in1=rz0[:, NSH:2 * NSH])
                nc.vector.tensor_add(out=h0[:], in0=ns_[:], in1=hmn[:])
                # --- L1 step t (input = updated h0) ---
                g1p = psC.tile([P, 512], FP, tag="pC")
                for gate in range(2):   # r, z: input+hidden summed in psum
                    nc.tensor.matmul(g1p[:, gate * P:(gate + 1) * P],
                                     lhsT=wih1[gate][:], rhs=h0[:],
                                     start=True, stop=False)
                    nc.tensor.matmul(g1p[:, gate * P:(gate + 1) * P],
                                     lhsT=whh1[gate][:], rhs=h1[:],
                                     start=False, stop=True)
                nc.tensor.matmul(g1p[:, 2 * P:3 * P], lhsT=wih1[2][:],
                                 rhs=h0[:], start=True, stop=True)
                nc.tensor.matmul(g1p[:, 3 * P:4 * P], lhsT=whh1[2][:],
                                 rhs=h1[:], start=True, stop=True)
                rz1 = pool.tile([P, 2 * NSH], FP, tag="rz1")
                nc.scalar.activation(rz1[:], g1p[:, 0:2 * P], AF.Sigmoid)
                n1 = pool.tile([P, NSH], FP, tag="n1")
                nc.vector.tensor_mul(out=n1[:], in0=rz1[:, 0:NSH],
                                     in1=g1p[:, 3 * P:4 * P])
                nc.vector.tensor_add(out=n1[:], in0=n1[:], in1=g1p[:, 2 * P:3 * P])
                nc.scalar.activation(n1[:], n1[:], AF.Tanh)
                hm1 = pool.tile([P, NSH], FP, tag="hm1")
                nc.vector.tensor_sub(out=hm1[:], in0=h1[:], in1=n1[:])
                nc.vector.tensor_mul(out=hm1[:], in0=hm1[:], in1=rz1[:, NSH:2 * NSH])
                nc.vector.tensor_add(out=h1[:], in0=n1[:], in1=hm1[:])
            hlast = h1

            lp = psA.tile([P, P], FP, tag="pA")
            nc.tensor.matmul(lp[:12, :NSH], lhsT=dec[:], rhs=hlast[:],
                             start=True, stop=True)
            lo = pool.tile([12, NSH], FP, tag="lo")
            nc.vector.tensor_mul(out=lo[:], in0=lp[:12, :NSH], in1=am[:])
            nc.sync.dma_start(out=out_d[:], in_=lo[:])

    nc.compile()
    return nc


def make_in_map(c, meta, W):
    """Per-core input arrays for run_bass_kernel_spmd."""
    A = c["arrays"]
    bf = lambda a: np.ascontiguousarray(a, dtype=BF16)
    f32 = lambda a: np.ascontiguousarray(a, dtype=np.float32)

    w1s = np.zeros((P, F1), np.float32)
    w1s[0:KSLOT * F_IN] = np.tile(f32(W["c1_wrel"]), (KSLOT, 1))
    w1s[KSLOT * F_IN:SLOT_W] = f32(W["c1_wroot"])
    wih0 = np.stack([np.stack([f32(W["w_ih0"])[g * P:(g + 1) * P, k * P:(k + 1) * P].T
                               for k in range(2)]) for g in range(3)])
    whh0 = np.stack([f32(W["w_hh0"])[g * P:(g + 1) * P, :].T for g in range(3)])
    wih1 = np.stack([f32(W["w_ih1"])[g * P:(g + 1) * P, :].T for g in range(3)])
    whh1 = np.stack([f32(W["w_hh1"])[g * P:(g + 1) * P, :].T for g in range(3)])
    amask = np.broadcast_to(A["amask"][None, :], (12, meta["n_shot_core"]))

    return {
        "xe1": A["xe1"],
        "xe2": A["xe2"],
        "w1s": bf(w1s),
        "s2T": A["s2T"],
        "s3T": A["s3T"],
        "pgT": A["pgT"],
        "amask": f32(amask),
        "ident": bf(np.eye(P, dtype=np.float32)),
        "wrel2": bf(W["c2_wrel"]),
        "wroot2": bf(W["c2_wroot"]),
        "wih0": bf(wih0),
        "whh0": bf(whh0),
        "wih1": bf(wih1),
        "whh1": bf(whh1),
        "dec": bf(W["dec_w"]),
    }


# ------------------------------------------------------------------
_CACHE = {}


def _get_nc(meta):
    key = (meta["NT_V"], meta["E2_slots"], meta["G_rows"],
           tuple(meta["T_sub"]))
    if key not in _CACHE:
        _CACHE[key] = build(meta, num_devices=NC)
    return _CACHE[key]


def kernel(**inputs):
    import sys as _sys
    if "/opt/trn_rl_repo" not in _sys.path:
        _sys.path.insert(0, "/opt/trn_rl_repo")
    from concourse.bass_utils import run_bass_kernel_spmd

    for k in ("c1_b", "c2_b", "b_ih0", "b_hh0", "b_ih1", "b_hh1", "dec_b",
              "empty_emb"):
        assert not np.any(np.asarray(inputs[k])), f"nonzero {k} unsupported"

    cores, meta = prep(inputs)
    W = {k: np.asarray(v, np.float32) for k, v in inputs.items()
         if k not in ("x", "edge_index", "edge_attr", "batch_labels",
                      "label_map", "B")}
    nc = _get_nc(meta)
    in_maps = [make_in_map(c, meta, W) for c in cores]
    res = None
    for attempt in range(6):
        try:
            res = run_bass_kernel_spmd(nc, in_maps, core_ids=list(range(NC)))
            break
        except Exception:
            if attempt == 5:
                raise
    global LAST_RES
    LAST_RES = res
    B = meta["B"]
    out = np.zeros((B, 12), np.float32)
    nsh = meta["n_shot_core"]
    for d in range(NC):
        lg = res.results[d]["out"]          # [12, nsh]
        s = d + NC * np.arange(nsh)
        out[s[s < B]] = lg.T[s < B]
    return out
